# revision 35
# baseline (speedup 1.0000x reference)
"""GQA attention with sliding-window+sink KV slicing on 8 trn2 NeuronCores.

Sharding: core = (batch b in 0..1, query-chunk c in 0..3); each core handles
1024 query tokens of one batch against the full sliced KV (sink 4 + window
1024 = 1028 positions), weights replicated.  No collectives; host concats.

v2 restructure (vs v0 phase-serial kernel):
  * K/V projections computed DIRECTLY TRANSPOSED (lhsT = W^T tiles, rhs =
    x^T) -- no PE transposes.  RoPE in the transposed [d, t] layout via a
    32-row-block swap matmul on PE + 3 bf16 DVE ops with sign-baked sin
    tables.
  * Q projection + RoPE fused INTO the attention head-pair loop (software
    pipelined one iteration ahead), so PE stays busy during the ACT-bound
    exp stretches and the HAM clock never re-throttles.
  * Causal masking split: fully-masked (tc,j) blocks are killed for free by
    a per-partition -1e30 bias table fed to the exp activation; only the 9
    diagonal-capable blocks get a DVE mask multiply (vs 18 before).
  * Softmax denominator via the ones-column-in-V trick (row 64 of the PV
    accumulator); per-head-pair normalization with reciprocal_approx_fast
    and a tiny selector matmul, all inside the loop.
  * Output projection last, PE-dense, weights streamed.
"""

import numpy as np
import ml_dtypes

import concourse.bass as bass
import concourse.bacc as bacc
import concourse.tile as tile
import concourse.mybir as mybir
from concourse.bass_utils import run_bass_kernel_spmd

BF = mybir.dt.bfloat16
F32 = mybir.dt.float32
BF_NP = ml_dtypes.bfloat16

# problem constants
D_MODEL = 2048
N_HEADS = 32
N_KV = 8
D_HEAD = 64
GROUP = 4
B, T = 2, 4096
WINDOW = 1024
SINK = 4
ROPE_BASE = 10000.0

# sharding/tiling constants
NCORES = 8
TQ = 1024             # query tokens per core
S = SINK + WINDOW     # 1028 kv positions
DMT = D_MODEL // 128  # 16 contraction tiles
TC = TQ // 512        # 2 query 512-chunks
QC = D_MODEL // 512   # 4 out-proj 512-chunks
ST = (S + 127) // 128  # 9 s-tiles (last has 4 rows)
HP = N_HEADS // 2     # 16 head pairs
KT = N_KV // 2        # 4 kv-head pair tiles
SCALE = float(1.0 / np.sqrt(D_HEAD))
NEG = -1.0e30
F8 = mybir.dt.float8e4
F8_NP = ml_dtypes.float8_e4m3fn
WSC = 64.0              # fp8 weight pre-scale for wq/wk (and wv, folded
                        # into wp host-side); un-done in the exp scale
ESCALE = SCALE / (WSC * WSC)  # exp scale: q,k each carry a WSC factor
DR = mybir.MatmulPerfMode.DoubleRow

_CACHED = {}


SP8 = 1040  # fp8 xkv dm-stride, padded so the DoubleRow pair-step is 16-aligned


def _sp(j):
    return 128 if j < ST - 1 else S - 128 * (ST - 1)


def _ins0(ap, dim_idx, n):
    """Copy of `ap` with a step-0 (broadcast) dim inserted at free-dim
    position `dim_idx` (0 = right after the partition dim)."""
    dims = list(ap.ap)
    dims.insert(1 + dim_idx, [0, n])
    return bass.AP(tensor=ap.tensor, offset=ap.offset, ap=dims)


def _build_bass():
    nc = bacc.Bacc("TRN2", target_bir_lowering=False, debug=False,
                   num_devices=NCORES)

    def din(name, shape, dt=BF):
        return nc.dram_tensor(name, shape, dt, kind="ExternalInput").ap()

    XQ = din("xq_t", [128, DMT * TQ])
    XKV = din("xkv_t", [128, DMT * S])
    WQ = din("wq_t", [128, HP * DMT * 128])
    WK = din("wk_t", [128, KT * DMT * 128])
    WV = din("wv_t", [128, DMT * 512])
    WP = din("wp_t", [128, QC * DMT * 512])
    COSQ = din("cosq_t", [128, TQ])
    SSINQ = din("ssinq_t", [128, TQ])
    COSK = din("cosk_t", [128, S])
    SSINK = din("ssink_t", [128, S])
    BIAS = din("bias_t", [128, 2 * ST], F32)
    MASK = din("mask_t", [128, 9 * 512])
    SEL = din("sel_t", [2, 128])
    OUT = nc.dram_tensor("out", [TQ, D_MODEL], F32, kind="ExternalOutput").ap()

    with tile.TileContext(nc) as tc:
        _body(tc, XQ, XKV, WQ, WK, WV, WP, COSQ, SSINQ, COSK, SSINK, BIAS,
              MASK, SEL, OUT)
    nc.compile()
    return nc


def _body(tc, XQ, XKV, WQ, WK, WV, WP, COSQ, SSINQ, COSK, SSINK, BIAS,
          MASK, SEL, OUT):
    nc = tc.nc

    def load(pool, name, src, shape, dt=BF):
        t = pool.tile(shape, dt, tag=name, name=name)
        nc.sync.dma_start(out=t[:, :], in_=src)
        return t

    with (
        tc.tile_pool(name="life", bufs=1) as life,
        tc.tile_pool(name="spp", bufs=2, space="PSUM") as spp,
        tc.tile_pool(name="psc", bufs=2, space="PSUM") as psc,
        tc.tile_pool(name="ppv", bufs=1, space="PSUM") as ppv,
    ):
        kT_sb = life.tile([128, N_KV * S], BF, tag="kT")   # [kd dup-halves, s]
        v_sb = life.tile([128, ST * N_KV * 65 + 63], BF, tag="v")
        ctx_sb = life.tile([128, HP * TQ], BF, tag="ctx")  # [dm, t]

        wqs = tc.alloc_tile_pool(name="wqs", bufs=2)
        wblks = {}
        wqh = [None] * HP

        def wq_dma(hp):
            wq_hp = wqs.tile([128, DMT * 128], BF, tag="wq")
            nc.sync.dma_start(
                out=wq_hp[:, :],
                in_=WQ[:, hp * DMT * 128:(hp + 1) * DMT * 128])
            wqh[hp] = wq_hp

        # ============ phase B: K/V projection (transposed) ================
        with (
            tc.tile_pool(name="p2s", bufs=1) as p2s,
            tc.tile_pool(name="kst", bufs=2) as kst,
        ):
            wk_sb = p2s.tile([128, KT * DMT * 128], BF, tag="wk_sb")
            for c in range(2):
                h = KT * DMT * 64
                nc.sync.dma_start(out=wk_sb[:, c * h:(c + 1) * h],
                                  in_=WK[:, c * h:(c + 1) * h])
            xkv_sb = p2s.tile([128, DMT * S], BF, tag="xkv_sb")
            for c in range(4):
                nc.sync.dma_start(
                    out=xkv_sb[:, c * 4 * S:(c + 1) * 4 * S],
                    in_=XKV[:, c * 4 * S:(c + 1) * 4 * S])
            xq_sb = life.tile([128, DMT * TQ], BF, tag="xq_sb")
            for c in range(4):
                nc.sync.dma_start(
                    out=xq_sb[:, c * 4 * TQ:(c + 1) * 4 * TQ],
                    in_=XQ[:, c * 4 * TQ:(c + 1) * 4 * TQ])
            wq_dma(0)
            wq_dma(1)
            wv_sb = load(p2s, "wv_sb", WV, [128, DMT * 512])
            cosk_sb = load(p2s, "cosk_sb", COSK, [128, S])
            ssink_sb = load(p2s, "ssink_sb", SSINK, [128, S])
            cosq_sb = load(life, "cosq_sb", COSQ, [128, TQ])
            ssinq_sb = load(life, "ssinq_sb", SSINQ, [128, TQ])
            sel_sb = load(life, "sel_sb", SEL, [2, 128])
            bias_sb = load(life, "bias_sb", BIAS, [128, 2 * ST], F32)
            mask_sb = load(life, "mask_sb", MASK, [128, 9 * 512])
            xkv3 = xkv_sb[:, :].rearrange("p (k s) -> p k s", k=DMT)
            wk3 = wk_sb[:, :].rearrange("p (k o) -> p k o", k=KT * DMT)
            wv3 = wv_sb[:, :].rearrange("p (k o) -> p k o", k=DMT)

            SCH = [(0, 512), (512, 512), (1024, 4)]
            for ot in range(KT):
                # kT o-tile [128 = heads (2ot, 2ot+1) x 64d, s]
                ka = kst.tile([128, S], BF, tag="ka")
                for off, w in SCH:
                    pk = spp.tile([128, 512], F32, tag="sp")
                    for dm in range(DMT):
                        nc.tensor.matmul(
                            pk[:, :w],
                            lhsT=wk3[:, ot * DMT + dm, :],
                            rhs=xkv3[:, dm, off:off + w],
                            start=(dm == 0), stop=(dm == DMT - 1))
                    nc.vector.tensor_copy(ka[:, off:off + w], pk[:, :w])
                kb = kst.tile([128, S], BF, tag="kb")
                for blk in range(4):
                    dst = 32 * blk
                    srcp = 32 * (blk ^ 1)
                    nc.sync.dma_start(out=kb[dst:dst + 32, :],
                                      in_=ka[srcp:srcp + 32, :])
                kc = kst.tile([128, S], BF, tag="kc")
                nc.vector.tensor_mul(kc[:, :], ka[:, :], cosk_sb[:, :])
                nc.vector.tensor_mul(kb[:, :], kb[:, :], ssink_sb[:, :])
                gA, gB = 2 * ot, 2 * ot + 1
                nc.vector.tensor_add(kT_sb[0:64, gA * S:(gA + 1) * S],
                                     kc[0:64, :], kb[0:64, :])
                nc.vector.tensor_add(kT_sb[64:128, gB * S:(gB + 1) * S],
                                     kc[64:128, :], kb[64:128, :])
            # duplicate each kv head's k rows into the other partition half
            for g in range(N_KV):
                if g % 2 == 0:
                    nc.sync.dma_start(out=kT_sb[64:128, g * S:(g + 1) * S],
                                      in_=kT_sb[0:64, g * S:(g + 1) * S])
                else:
                    nc.sync.dma_start(out=kT_sb[0:64, g * S:(g + 1) * S],
                                      in_=kT_sb[64:128, g * S:(g + 1) * S])

            # V projection (+ ones columns), s-major as before
            nc.vector.memset(v_sb[:, :], 0.0)
            v4 = v_sb[:, 0:ST * N_KV * 65].rearrange("p (s h c) -> p s h c", s=ST, c=65)
            for ss in range(ST):
                sp = _sp(ss)
                pv = spp.tile([128, 512], F32, tag="sp")
                for dm in range(DMT):
                    nc.tensor.matmul(
                        pv[:sp, :],
                        lhsT=xkv3[:, dm, ss * 128:ss * 128 + sp],
                        rhs=wv3[:, dm, :],
                        start=(dm == 0), stop=(dm == DMT - 1))
                nc.vector.tensor_copy(
                    v4[:sp, ss, :, 0:64],
                    pv[:sp, :].rearrange("p (h d) -> p h d", h=8))
                nc.vector.memset(v4[:sp, ss, :, 64:65], 1.0)

        # ============ fused Q-proj + attention head-pair loop =============
        wst2 = tc.alloc_tile_pool(name="wst2", bufs=2)

        def wp_dma(ob):
            for h in range(2):
                wb = wst2.tile([128, 8 * 512], BF, tag="wblk2", bufs=3,
                               name=f"wb{ob}{h}")
                nc.sync.dma_start(
                    out=wb[:, :],
                    in_=WP[:, (ob * DMT + h * 8) * 512:
                            (ob * DMT + h * 8 + 8) * 512])
                wblks[(ob, h)] = wb

        with (
            tc.tile_pool(name="qtp", bufs=2) as qtp,
            tc.tile_pool(name="prb", bufs=2) as prb,
            tc.tile_pool(name="stg", bufs=2) as stg,
            tc.tile_pool(name="dnp", bufs=2) as dnp,
        ):
            mask3 = mask_sb[:, :].rearrange("p (m v) -> p m v", m=9)
            qts = [None] * HP
            denh = [None] * HP

            xq3 = xq_sb[:, :].rearrange("p (k t) -> p k t", k=DMT)

            def qproj(hp):
                wq3 = wqh[hp][:, :].rearrange("p (k o) -> p k o", k=DMT)
                qa = qtp.tile([128, TQ], BF, tag="qa")
                for ch in range(TC):
                    pq = spp.tile([128, 512], F32, tag="sp")
                    for dm in range(DMT):
                        nc.tensor.matmul(
                            pq[:, :],
                            lhsT=wq3[:, dm, :],
                            rhs=xq3[:, dm, ch * 512:ch * 512 + 512],
                            start=(dm == 0), stop=(dm == DMT - 1))
                    nc.vector.tensor_copy(qa[:, ch * 512:(ch + 1) * 512],
                                          pq[:, :])
                qb = qtp.tile([128, TQ], BF, tag="qb", bufs=1)
                for blk in range(4):
                    dst = 32 * blk
                    srcp = 32 * (blk ^ 1)
                    nc.sync.dma_start(out=qb[dst:dst + 32, :],
                                      in_=qa[srcp:srcp + 32, :])
                qt = qtp.tile([128, TQ], BF, tag="qt")
                nc.vector.tensor_mul(qt[:, :], qa[:, :], cosq_sb[:, :])
                nc.vector.tensor_mul(qb[:, :], qb[:, :], ssinq_sb[:, :])
                nc.vector.tensor_add(qt[:, :], qt[:, :], qb[:, :])
                qts[hp] = qt

            def attn_core(hp):
                qt = qts[hp]
                g = hp // 2
                dens_hp = dnp.tile([2, TQ], BF, tag="dh", bufs=2)
                denh[hp] = dens_hp
                for tcq in range(TC):
                    probs = prb.tile([128, ST * 1024], BF, tag="probs")
                    p4 = probs[:, :].rearrange("p (s u v) -> p s u v",
                                               s=ST, u=2)
                    for j in range(ST):
                        sp = _sp(j)
                        sc = psc.tile([128, 1024], F32, tag="sc")
                        nc.tensor.matmul(
                            sc[:sp, 0:512],
                            lhsT=kT_sb[0:64, g * S + j * 128:
                                       g * S + j * 128 + sp],
                            rhs=qt[0:64, tcq * 512:tcq * 512 + 512],
                            start=True, stop=True, tile_position=(0, 0))
                        nc.tensor.matmul(
                            sc[:sp, 512:1024],
                            lhsT=kT_sb[64:128, g * S + j * 128:
                                       g * S + j * 128 + sp],
                            rhs=qt[64:128, tcq * 512:tcq * 512 + 512],
                            start=True, stop=True, tile_position=(64, 0))
                        nc.scalar.activation(
                            p4[:sp, j, :, :],
                            sc[:sp, :].rearrange("p (u v) -> p u v", u=2),
                            mybir.ActivationFunctionType.Exp, scale=SCALE,
                            bias=bias_sb[:sp, tcq * ST + j:tcq * ST + j + 1])
                    # data-mask only diagonal-capable blocks; the rest were
                    # killed (or kept) by the exp bias
                    if tcq == 0:
                        nc.vector.tensor_mul(
                            p4[:, 0:4, :, :], p4[:, 0:4, :, :],
                            _ins0(mask3[:, 0:4, :], 1, 2))
                        spl = _sp(ST - 1)
                        nc.vector.tensor_mul(
                            p4[:spl, 8, :, :], p4[:spl, 8, :, :],
                            _ins0(mask3[:spl, 8, :], 0, 2))
                    else:
                        nc.vector.tensor_mul(
                            p4[:, 4:8, :, :], p4[:, 4:8, :, :],
                            _ins0(mask3[:, 4:8, :], 1, 2))
                    # PV: ctx_aug[65, t] per head; denominator in row 64
                    pvt = ppv.tile([128, 1024], F32, tag="pv")
                    for j in range(ST):
                        sp = _sp(j)
                        vw = v_sb[:sp, (j * N_KV + g) * 65:
                                  (j * N_KV + g) * 65 + 128]
                        nc.tensor.matmul(
                            pvt[0:128, 0:512],
                            lhsT=vw,
                            rhs=p4[:sp, j, 0, :],
                            start=(j == 0), stop=(j == ST - 1))
                        nc.tensor.matmul(
                            pvt[0:128, 512:1024],
                            lhsT=vw,
                            rhs=p4[:sp, j, 1, :],
                            start=(j == 0), stop=(j == ST - 1))
                    # ctx head A plain copy (same partitions)
                    nc.vector.tensor_copy(
                        ctx_sb[0:64, hp * TQ + tcq * 512:
                               hp * TQ + tcq * 512 + 512],
                        pvt[0:64, 0:512])
                    # head B ctx + dens staged; DMA shifts partitions
                    stb = stg.tile([128, 1024], BF, tag="stb")
                    nc.vector.tensor_copy(stb[0:65, 0:512],
                                          pvt[0:65, 512:1024])
                    nc.vector.tensor_copy(stb[64:65, 512:1024],
                                          pvt[64:65, 0:512])
                    nc.sync.dma_start(
                        out=ctx_sb[64:128, hp * TQ + tcq * 512:
                                   hp * TQ + tcq * 512 + 512],
                        in_=stb[0:64, 0:512])
                    nc.sync.dma_start(
                        out=dens_hp[0:2, tcq * 512:(tcq + 1) * 512],
                        in_=stb[64:65, 0:1024])
                qts[hp] = None

            def norm(hp):
                # normalize this head pair's ctx (decoupled by 2 iterations
                # so the reciprocal chain never stalls the score pipeline)
                dens_hp = denh[hp]
                densf = dnp.tile([2, TQ], F32, tag="df", bufs=1)
                nc.vector.tensor_copy(densf[:, :], dens_hp[:, :])
                rf = dnp.tile([2, TQ], F32, tag="rf", bufs=1)
                nc.vector.reciprocal_approx_fast(rf[:, :], densf[:, :])
                rb = dnp.tile([2, TQ], BF, tag="rb", bufs=1)
                nc.vector.tensor_copy(rb[:, :], rf[:, :])
                pr = psc.tile([128, 1024], F32, tag="sc")
                for ch in range(TC):
                    nc.tensor.matmul(pr[:, ch * 512:(ch + 1) * 512],
                                     lhsT=sel_sb[:, :],
                                     rhs=rb[:, ch * 512:(ch + 1) * 512],
                                     start=True, stop=True)
                csl = ctx_sb[:, hp * TQ:(hp + 1) * TQ]
                nc.vector.tensor_mul(csl, csl, pr[:, :])
                denh[hp] = None

            pos = {}
            for i in range(HP + 2):
                if 2 <= i < HP:
                    wq_dma(i)
                if i == HP - 2:
                    wp_dma(0)
                if i == HP - 1:
                    wp_dma(1)
                if 1 <= i <= HP:
                    attn_core(i - 1)
                if i < HP:
                    qproj(i)
                if i == HP:
                    # fill the norm-chain tail with the first out-proj
                    # accumulations over the already-normalized head pairs
                    for tt in range(2):
                        po = spp.tile([128, 512], F32, tag="sp")
                        for hp in range(HP - 2):
                            nc.tensor.matmul(
                                po[:, :],
                                lhsT=ctx_sb[:, hp * TQ + tt * 128:
                                            hp * TQ + tt * 128 + 128],
                                rhs=wblks[(0, hp // 8)][:, (hp % 8) * 512:
                                                        (hp % 8) * 512 + 512],
                                start=(hp == 0), stop=False)
                        pos[tt] = po
                if i >= 2:
                    norm(i - 2)

        # ============ phase E: output projection ==========================
        with (
            tc.tile_pool(name="osb", bufs=3) as osb,
        ):
            for ob in range(QC):
                if ob >= 2:
                    wp_dma(ob)
                for tt in range(TQ // 128):
                    if ob == 0 and tt in pos:
                        po = pos[tt]
                        hps = range(HP - 2, HP)
                    else:
                        po = spp.tile([128, 512], F32, tag="sp")
                        hps = range(HP)
                    for hp in hps:
                        nc.tensor.matmul(
                            po[:, :],
                            lhsT=ctx_sb[:, hp * TQ + tt * 128:
                                        hp * TQ + tt * 128 + 128],
                            rhs=wblks[(ob, hp // 8)][:, (hp % 8) * 512:
                                                     (hp % 8) * 512 + 512],
                            start=(hp == 0 and (ob != 0 or tt not in pos)),
                            stop=(hp == HP - 1))
                    ot = osb.tile([128, 512], F32, tag="outsb")
                    nc.vector.tensor_copy(ot[:, :], po[:, :])
                    nc.sync.dma_start(
                        out=OUT[tt * 128:(tt + 1) * 128,
                                ob * 512:(ob + 1) * 512],
                        in_=ot[:, :])
        wst2.release()
        wqs.release()


# ---------------------------------------------------------------------------
# host-side data prep
# ---------------------------------------------------------------------------

def _tile_weight_T(w, nt):
    # w [nt*128, 2048] -> [128, nt*DMT*128]:
    # [p][t, k, o] = w[128*t + o, 128*k + p]
    return np.ascontiguousarray(
        w.reshape(nt, 128, DMT, 128).transpose(3, 0, 2, 1).reshape(128, -1)
    ).astype(BF_NP)


def _tile_weight_kv(w):
    # w [512, 2048] -> [128, DMT*512]: [p][k,o] = w[o, dmt*128+p]
    return np.ascontiguousarray(
        w.reshape(512, DMT, 128).transpose(2, 1, 0).reshape(128, -1)
    ).astype(BF_NP)


def _tile_weight_q(w):
    # w [2048, 2048] -> [128, QC*DMT*512]: [p][qc,dmt,o] = w[qc*512+o, dmt*128+p]
    return np.ascontiguousarray(
        w.reshape(QC, 512, DMT, 128).transpose(3, 0, 2, 1).reshape(128, -1)
    ).astype(BF_NP)


def _tile_x(xt):
    # xt [ntok, 2048] -> [128, DMT*ntok]: [p][k,t] = xt[t, 128k+p]
    n = xt.shape[0]
    return np.ascontiguousarray(
        xt.T.reshape(DMT, 128, n).transpose(1, 0, 2).reshape(128, -1)
    ).astype(BF_NP)


def _rope_tables_T(pos):
    # transposed-layout tables [128, len(pos)]: row r = 64*a + d (a head in
    # pair, d dim); cos[r, t] = cos(pos_t * invf[d % 32]);
    # ssin[r, t] = sin(...) * (-1 if d < 32 else +1)
    invf = 1.0 / (ROPE_BASE ** (np.arange(0, D_HEAD, 2, dtype=np.float64)
                                / D_HEAD))
    d = np.arange(128) % 64
    ang = pos[None, :] * invf[d % 32][:, None]          # [128, n]
    sign = np.where(d < 32, -1.0, 1.0)[:, None]
    cos = np.cos(ang).astype(BF_NP)
    ssin = (np.sin(ang) * sign).astype(BF_NP)
    return np.ascontiguousarray(cos), np.ascontiguousarray(ssin)


# data-masked block list: column m of the mask tensor covers (tc_m, j_m)
_MASK_POS = [(0, 0), (0, 1), (0, 2), (0, 3),
             (1, 4), (1, 5), (1, 6), (1, 7), (0, 8)]


def _core_inputs(x, shared, b, c):
    qoff = c * TQ
    xq = x[b, qoff:qoff + TQ]
    xkv = np.concatenate([x[b, :SINK], x[b, T - WINDOW:]], 0)

    qpos = (qoff + np.arange(TQ)).astype(np.float64)
    kpos = np.concatenate([np.arange(SINK),
                           np.arange(T - WINDOW, T)]).astype(np.float64)
    cosq, ssinq = _rope_tables_T(qpos)
    cosk, ssink = _rope_tables_T(kpos)

    # exp bias [128, tc*9+j]: -1e30 where the whole row is masked
    p = np.arange(128)
    bias = np.zeros((128, 2 * ST), np.float32)
    for tcq in range(TC):
        for j in range(ST):
            kill = (128 * j + p > qoff + 512 * tcq + 511) | (128 * j + p >= S)
            bias[kill, tcq * ST + j] = NEG

    # data mask [128, 9 positions, 512]
    mask = np.zeros((128, 9, 512), BF_NP)
    t = np.arange(512)
    for m, (tcq, j) in enumerate(_MASK_POS):
        keep = (qoff + 512 * tcq + t[None, :]) >= (128 * j + p[:, None])
        mask[:, m, :] = keep.astype(BF_NP)

    d = {
        "xq_t": _tile_x(xq),
        "xkv_t": _tile_x(xkv),
        "cosq_t": cosq, "ssinq_t": ssinq,
        "cosk_t": cosk, "ssink_t": ssink,
        "bias_t": bias,
        "mask_t": np.ascontiguousarray(mask.reshape(128, -1)),
    }
    d.update(shared)
    return d


def _prep_all(x, wq, wk, wv, w_proj):
    # dens rows: row 0 = den(head B), row 1 = den(head A) (stage swizzle)
    sel = np.zeros((2, 128), dtype=BF_NP)
    sel[0, 64:128] = 1
    sel[1, 0:64] = 1
    shared = {
        "wq_t": _tile_weight_T(wq, HP),
        "wk_t": _tile_weight_T(wk, KT),
        "wv_t": _tile_weight_kv(wv),
        "wp_t": _tile_weight_q(w_proj),
        "sel_t": sel,
    }
    return [_core_inputs(x, shared, *divmod(core, 4)) for core in range(NCORES)]


def _get_nc():
    if "nc" not in _CACHED:
        _CACHED["nc"] = _build_bass()
    return _CACHED["nc"]


def _run(x, wq, wk, wv, w_proj, trace=False, **kw):
    nc = _get_nc()
    in_maps = _prep_all(np.asarray(x, np.float32), np.asarray(wq, np.float32),
                        np.asarray(wk, np.float32), np.asarray(wv, np.float32),
                        np.asarray(w_proj, np.float32))
    res = run_bass_kernel_spmd(nc, in_maps, list(range(NCORES)), trace=trace,
                               **kw)
    out = np.empty((B, T, D_MODEL), np.float32)
    for core in range(NCORES):
        b, c = divmod(core, 4)
        out[b, c * TQ:(c + 1) * TQ] = np.asarray(res.results[core]["out"],
                                                 np.float32)
    return out, res


def kernel(x, wq, wk, wv, w_proj):
    out, _ = _run(x, wq, wk, wv, w_proj)
    return out


# revision 36
# speedup vs baseline: 1.0105x; 1.0105x over previous
"""GQA attention with sliding-window+sink KV slicing on 8 trn2 NeuronCores.

Sharding: core = (batch b in 0..1, query-chunk c in 0..3); each core handles
1024 query tokens of one batch against the full sliced KV (sink 4 + window
1024 = 1028 positions), weights replicated.  No collectives; host concats.

v2 restructure (vs v0 phase-serial kernel):
  * K/V projections computed DIRECTLY TRANSPOSED (lhsT = W^T tiles, rhs =
    x^T) -- no PE transposes.  RoPE in the transposed [d, t] layout via a
    32-row-block swap matmul on PE + 3 bf16 DVE ops with sign-baked sin
    tables.
  * Q projection + RoPE fused INTO the attention head-pair loop (software
    pipelined one iteration ahead), so PE stays busy during the ACT-bound
    exp stretches and the HAM clock never re-throttles.
  * Causal masking split: fully-masked (tc,j) blocks are killed for free by
    a per-partition -1e30 bias table fed to the exp activation; only the 9
    diagonal-capable blocks get a DVE mask multiply (vs 18 before).
  * Softmax denominator via the ones-column-in-V trick (row 64 of the PV
    accumulator); per-head-pair normalization with reciprocal_approx_fast
    and a tiny selector matmul, all inside the loop.
  * Output projection last, PE-dense, weights streamed.
"""

import numpy as np
import ml_dtypes

import concourse.bass as bass
import concourse.bacc as bacc
import concourse.tile as tile
import concourse.mybir as mybir
from concourse.bass_utils import run_bass_kernel_spmd

BF = mybir.dt.bfloat16
F32 = mybir.dt.float32
BF_NP = ml_dtypes.bfloat16

# problem constants
D_MODEL = 2048
N_HEADS = 32
N_KV = 8
D_HEAD = 64
GROUP = 4
B, T = 2, 4096
WINDOW = 1024
SINK = 4
ROPE_BASE = 10000.0

# sharding/tiling constants
NCORES = 8
TQ = 1024             # query tokens per core
S = SINK + WINDOW     # 1028 kv positions
DMT = D_MODEL // 128  # 16 contraction tiles
TC = TQ // 512        # 2 query 512-chunks
QC = D_MODEL // 512   # 4 out-proj 512-chunks
ST = (S + 127) // 128  # 9 s-tiles (last has 4 rows)
HP = N_HEADS // 2     # 16 head pairs
KT = N_KV // 2        # 4 kv-head pair tiles
SCALE = float(1.0 / np.sqrt(D_HEAD))
NEG = -1.0e30
F8 = mybir.dt.float8e4
F8_NP = ml_dtypes.float8_e4m3fn
WSC = 64.0              # fp8 weight pre-scale for wq/wk (and wv, folded
                        # into wp host-side); un-done in the exp scale
ESCALE = SCALE / (WSC * WSC)  # exp scale: q,k each carry a WSC factor
DR = mybir.MatmulPerfMode.DoubleRow

_CACHED = {}


SP8 = 1040  # fp8 xkv dm-stride, padded so the DoubleRow pair-step is 16-aligned


def _sp(j):
    return 128 if j < ST - 1 else S - 128 * (ST - 1)


def _ins0(ap, dim_idx, n):
    """Copy of `ap` with a step-0 (broadcast) dim inserted at free-dim
    position `dim_idx` (0 = right after the partition dim)."""
    dims = list(ap.ap)
    dims.insert(1 + dim_idx, [0, n])
    return bass.AP(tensor=ap.tensor, offset=ap.offset, ap=dims)


def _build_bass():
    nc = bacc.Bacc("TRN2", target_bir_lowering=False, debug=False,
                   num_devices=NCORES)

    def din(name, shape, dt=BF):
        return nc.dram_tensor(name, shape, dt, kind="ExternalInput").ap()

    XQ = din("xq_t", [128, DMT * TQ])
    XKV = din("xkv_t", [128, DMT * S])
    WQ = din("wq_t", [128, HP * DMT * 128])
    WK = din("wk_t", [128, KT * DMT * 128])
    WV = din("wv_t", [128, DMT * 512])
    WP = din("wp_t", [128, QC * DMT * 512])
    COSQ = din("cosq_t", [128, TQ])
    SSINQ = din("ssinq_t", [128, TQ])
    COSK = din("cosk_t", [128, S])
    SSINK = din("ssink_t", [128, S])
    BIAS = din("bias_t", [128, 2 * ST], F32)
    MASK = din("mask_t", [128, 9 * 512])
    SWAP = din("swap_t", [128, 128])
    SEL = din("sel_t", [2, 128])
    OUT = nc.dram_tensor("out", [TQ, D_MODEL], F32, kind="ExternalOutput").ap()

    with tile.TileContext(nc) as tc:
        _body(tc, XQ, XKV, WQ, WK, WV, WP, COSQ, SSINQ, COSK, SSINK, BIAS,
              MASK, SWAP, SEL, OUT)
    nc.compile()
    return nc


def _body(tc, XQ, XKV, WQ, WK, WV, WP, COSQ, SSINQ, COSK, SSINK, BIAS,
          MASK, SWAP, SEL, OUT):
    nc = tc.nc

    def load(pool, name, src, shape, dt=BF):
        t = pool.tile(shape, dt, tag=name, name=name)
        nc.sync.dma_start(out=t[:, :], in_=src)
        return t

    with (
        tc.tile_pool(name="life", bufs=1) as life,
        tc.tile_pool(name="spp", bufs=2, space="PSUM") as spp,
        tc.tile_pool(name="psc", bufs=2, space="PSUM") as psc,
        tc.tile_pool(name="ppv", bufs=1, space="PSUM") as ppv,
    ):
        kT_sb = life.tile([128, N_KV * S], BF, tag="kT")   # [kd dup-halves, s]
        v_sb = life.tile([128, ST * N_KV * 65 + 63], BF, tag="v")
        ctx_sb = life.tile([128, HP * TQ], BF, tag="ctx")  # [dm, t]

        wqs = tc.alloc_tile_pool(name="wqs", bufs=2)
        wblks = {}
        wqh = [None] * HP

        def wq_dma(hp):
            wq_hp = wqs.tile([128, DMT * 128], BF, tag="wq")
            nc.sync.dma_start(
                out=wq_hp[:, :],
                in_=WQ[:, hp * DMT * 128:(hp + 1) * DMT * 128])
            wqh[hp] = wq_hp

        # ============ phase B: K/V projection (transposed) ================
        with (
            tc.tile_pool(name="p2s", bufs=1) as p2s,
            tc.tile_pool(name="kst", bufs=2) as kst,
        ):
            wk_sb = p2s.tile([128, KT * DMT * 128], BF, tag="wk_sb")
            for c in range(2):
                h = KT * DMT * 64
                nc.sync.dma_start(out=wk_sb[:, c * h:(c + 1) * h],
                                  in_=WK[:, c * h:(c + 1) * h])
            xkv_sb = p2s.tile([128, DMT * S], BF, tag="xkv_sb")
            for c in range(4):
                nc.sync.dma_start(
                    out=xkv_sb[:, c * 4 * S:(c + 1) * 4 * S],
                    in_=XKV[:, c * 4 * S:(c + 1) * 4 * S])
            xq_sb = life.tile([128, DMT * TQ], BF, tag="xq_sb")
            for c in range(4):
                nc.sync.dma_start(
                    out=xq_sb[:, c * 4 * TQ:(c + 1) * 4 * TQ],
                    in_=XQ[:, c * 4 * TQ:(c + 1) * 4 * TQ])
            wq_dma(0)
            wq_dma(1)
            wv_sb = load(p2s, "wv_sb", WV, [128, DMT * 512])
            cosk_sb = load(p2s, "cosk_sb", COSK, [128, S])
            ssink_sb = load(p2s, "ssink_sb", SSINK, [128, S])
            cosq_sb = load(life, "cosq_sb", COSQ, [128, TQ])
            ssinq_sb = load(life, "ssinq_sb", SSINQ, [128, TQ])
            sel_sb = load(life, "sel_sb", SEL, [2, 128])
            swap_sb = load(life, "swap_sb", SWAP, [128, 128])
            bias_sb = load(life, "bias_sb", BIAS, [128, 2 * ST], F32)
            mask_sb = load(life, "mask_sb", MASK, [128, 9 * 512])
            xkv3 = xkv_sb[:, :].rearrange("p (k s) -> p k s", k=DMT)
            wk3 = wk_sb[:, :].rearrange("p (k o) -> p k o", k=KT * DMT)
            wv3 = wv_sb[:, :].rearrange("p (k o) -> p k o", k=DMT)

            SCH = [(0, 512), (512, 512), (1024, 4)]
            for ot in range(KT):
                # kT o-tile [128 = heads (2ot, 2ot+1) x 64d, s]
                ka = kst.tile([128, S], BF, tag="ka")
                for off, w in SCH:
                    pk = spp.tile([128, 512], F32, tag="sp")
                    for dm in range(DMT):
                        nc.tensor.matmul(
                            pk[:, :w],
                            lhsT=wk3[:, ot * DMT + dm, :],
                            rhs=xkv3[:, dm, off:off + w],
                            start=(dm == 0), stop=(dm == DMT - 1))
                    nc.vector.tensor_copy(ka[:, off:off + w], pk[:, :w])
                kb = kst.tile([128, S], BF, tag="kb")
                for off, w in SCH:
                    ps = spp.tile([128, 512], F32, tag="sp")
                    nc.tensor.matmul(ps[:, :w], lhsT=swap_sb[:, :],
                                     rhs=ka[:, off:off + w],
                                     start=True, stop=True)
                    nc.vector.tensor_copy(kb[:, off:off + w], ps[:, :w])
                kc = kst.tile([128, S], BF, tag="kc")
                nc.vector.tensor_mul(kc[:, :], ka[:, :], cosk_sb[:, :])
                nc.vector.tensor_mul(kb[:, :], kb[:, :], ssink_sb[:, :])
                gA, gB = 2 * ot, 2 * ot + 1
                nc.vector.tensor_add(kT_sb[0:64, gA * S:(gA + 1) * S],
                                     kc[0:64, :], kb[0:64, :])
                nc.vector.tensor_add(kT_sb[64:128, gB * S:(gB + 1) * S],
                                     kc[64:128, :], kb[64:128, :])
            # duplicate each kv head's k rows into the other partition half
            for g in range(N_KV):
                if g % 2 == 0:
                    nc.sync.dma_start(out=kT_sb[64:128, g * S:(g + 1) * S],
                                      in_=kT_sb[0:64, g * S:(g + 1) * S])
                else:
                    nc.sync.dma_start(out=kT_sb[0:64, g * S:(g + 1) * S],
                                      in_=kT_sb[64:128, g * S:(g + 1) * S])

            # V projection (+ ones columns), s-major as before
            nc.vector.memset(v_sb[:, :], 0.0)
            v4 = v_sb[:, 0:ST * N_KV * 65].rearrange("p (s h c) -> p s h c", s=ST, c=65)
            for ss in range(ST):
                sp = _sp(ss)
                pv = spp.tile([128, 512], F32, tag="sp")
                for dm in range(DMT):
                    nc.tensor.matmul(
                        pv[:sp, :],
                        lhsT=xkv3[:, dm, ss * 128:ss * 128 + sp],
                        rhs=wv3[:, dm, :],
                        start=(dm == 0), stop=(dm == DMT - 1))
                nc.vector.tensor_copy(
                    v4[:sp, ss, :, 0:64],
                    pv[:sp, :].rearrange("p (h d) -> p h d", h=8))
                nc.vector.memset(v4[:sp, ss, :, 64:65], 1.0)

        # ============ fused Q-proj + attention head-pair loop =============
        wst2 = tc.alloc_tile_pool(name="wst2", bufs=2)

        def wp_dma(ob):
            for h in range(2):
                wb = wst2.tile([128, 8 * 512], BF, tag="wblk2", bufs=3,
                               name=f"wb{ob}{h}")
                nc.sync.dma_start(
                    out=wb[:, :],
                    in_=WP[:, (ob * DMT + h * 8) * 512:
                            (ob * DMT + h * 8 + 8) * 512])
                wblks[(ob, h)] = wb

        with (
            tc.tile_pool(name="qtp", bufs=2) as qtp,
            tc.tile_pool(name="prb", bufs=2) as prb,
            tc.tile_pool(name="stg", bufs=2) as stg,
            tc.tile_pool(name="dnp", bufs=2) as dnp,
        ):
            mask3 = mask_sb[:, :].rearrange("p (m v) -> p m v", m=9)
            qts = [None] * HP
            denh = [None] * HP

            xq3 = xq_sb[:, :].rearrange("p (k t) -> p k t", k=DMT)

            def qproj(hp):
                wq3 = wqh[hp][:, :].rearrange("p (k o) -> p k o", k=DMT)
                qa = qtp.tile([128, TQ], BF, tag="qa")
                for ch in range(TC):
                    pq = spp.tile([128, 512], F32, tag="sp")
                    for dm in range(DMT):
                        nc.tensor.matmul(
                            pq[:, :],
                            lhsT=wq3[:, dm, :],
                            rhs=xq3[:, dm, ch * 512:ch * 512 + 512],
                            start=(dm == 0), stop=(dm == DMT - 1))
                    nc.vector.tensor_copy(qa[:, ch * 512:(ch + 1) * 512],
                                          pq[:, :])
                qb = qtp.tile([128, TQ], BF, tag="qb", bufs=1)
                for ch in range(TC):
                    ps = spp.tile([128, 512], F32, tag="sp")
                    nc.tensor.matmul(ps[:, :], lhsT=swap_sb[:, :],
                                     rhs=qa[:, ch * 512:(ch + 1) * 512],
                                     start=True, stop=True)
                    nc.vector.tensor_copy(qb[:, ch * 512:(ch + 1) * 512],
                                          ps[:, :])
                qt = qtp.tile([128, TQ], BF, tag="qt")
                nc.vector.tensor_mul(qt[:, :], qa[:, :], cosq_sb[:, :])
                nc.vector.tensor_mul(qb[:, :], qb[:, :], ssinq_sb[:, :])
                nc.vector.tensor_add(qt[:, :], qt[:, :], qb[:, :])
                qts[hp] = qt

            def attn_core(hp):
                qt = qts[hp]
                g = hp // 2
                dens_hp = dnp.tile([2, TQ], BF, tag="dh", bufs=2)
                denh[hp] = dens_hp
                for tcq in range(TC):
                    probs = prb.tile([128, ST * 1024], BF, tag="probs")
                    p4 = probs[:, :].rearrange("p (s u v) -> p s u v",
                                               s=ST, u=2)
                    for j in range(ST):
                        sp = _sp(j)
                        sc = psc.tile([128, 1024], F32, tag="sc")
                        nc.tensor.matmul(
                            sc[:sp, 0:512],
                            lhsT=kT_sb[0:64, g * S + j * 128:
                                       g * S + j * 128 + sp],
                            rhs=qt[0:64, tcq * 512:tcq * 512 + 512],
                            start=True, stop=True, tile_position=(0, 0))
                        nc.tensor.matmul(
                            sc[:sp, 512:1024],
                            lhsT=kT_sb[64:128, g * S + j * 128:
                                       g * S + j * 128 + sp],
                            rhs=qt[64:128, tcq * 512:tcq * 512 + 512],
                            start=True, stop=True, tile_position=(64, 0))
                        nc.scalar.activation(
                            p4[:sp, j, :, :],
                            sc[:sp, :].rearrange("p (u v) -> p u v", u=2),
                            mybir.ActivationFunctionType.Exp, scale=SCALE,
                            bias=bias_sb[:sp, tcq * ST + j:tcq * ST + j + 1])
                    # data-mask only diagonal-capable blocks; the rest were
                    # killed (or kept) by the exp bias
                    if tcq == 0:
                        nc.vector.tensor_mul(
                            p4[:, 0:4, :, :], p4[:, 0:4, :, :],
                            _ins0(mask3[:, 0:4, :], 1, 2))
                        spl = _sp(ST - 1)
                        nc.vector.tensor_mul(
                            p4[:spl, 8, :, :], p4[:spl, 8, :, :],
                            _ins0(mask3[:spl, 8, :], 0, 2))
                    else:
                        nc.vector.tensor_mul(
                            p4[:, 4:8, :, :], p4[:, 4:8, :, :],
                            _ins0(mask3[:, 4:8, :], 1, 2))
                    # PV: ctx_aug[65, t] per head; denominator in row 64
                    pvt = ppv.tile([128, 1024], F32, tag="pv")
                    for j in range(ST):
                        sp = _sp(j)
                        vw = v_sb[:sp, (j * N_KV + g) * 65:
                                  (j * N_KV + g) * 65 + 128]
                        nc.tensor.matmul(
                            pvt[0:128, 0:512],
                            lhsT=vw,
                            rhs=p4[:sp, j, 0, :],
                            start=(j == 0), stop=(j == ST - 1))
                        nc.tensor.matmul(
                            pvt[0:128, 512:1024],
                            lhsT=vw,
                            rhs=p4[:sp, j, 1, :],
                            start=(j == 0), stop=(j == ST - 1))
                    # ctx head A plain copy (same partitions)
                    nc.vector.tensor_copy(
                        ctx_sb[0:64, hp * TQ + tcq * 512:
                               hp * TQ + tcq * 512 + 512],
                        pvt[0:64, 0:512])
                    # head B ctx + dens staged; DMA shifts partitions
                    stb = stg.tile([128, 1024], BF, tag="stb")
                    nc.vector.tensor_copy(stb[0:65, 0:512],
                                          pvt[0:65, 512:1024])
                    nc.vector.tensor_copy(stb[64:65, 512:1024],
                                          pvt[64:65, 0:512])
                    nc.sync.dma_start(
                        out=ctx_sb[64:128, hp * TQ + tcq * 512:
                                   hp * TQ + tcq * 512 + 512],
                        in_=stb[0:64, 0:512])
                    nc.sync.dma_start(
                        out=dens_hp[0:2, tcq * 512:(tcq + 1) * 512],
                        in_=stb[64:65, 0:1024])
                qts[hp] = None

            def norm(hp):
                # normalize this head pair's ctx (decoupled by 2 iterations
                # so the reciprocal chain never stalls the score pipeline)
                dens_hp = denh[hp]
                densf = dnp.tile([2, TQ], F32, tag="df", bufs=1)
                nc.vector.tensor_copy(densf[:, :], dens_hp[:, :])
                rf = dnp.tile([2, TQ], F32, tag="rf", bufs=1)
                nc.vector.reciprocal_approx_fast(rf[:, :], densf[:, :])
                rb = dnp.tile([2, TQ], BF, tag="rb", bufs=1)
                nc.vector.tensor_copy(rb[:, :], rf[:, :])
                pr = psc.tile([128, 1024], F32, tag="sc")
                for ch in range(TC):
                    nc.tensor.matmul(pr[:, ch * 512:(ch + 1) * 512],
                                     lhsT=sel_sb[:, :],
                                     rhs=rb[:, ch * 512:(ch + 1) * 512],
                                     start=True, stop=True)
                csl = ctx_sb[:, hp * TQ:(hp + 1) * TQ]
                nc.vector.tensor_mul(csl, csl, pr[:, :])
                denh[hp] = None

            pos = {}
            for i in range(HP + 2):
                if 2 <= i < HP:
                    wq_dma(i)
                if i == HP - 2:
                    wp_dma(0)
                if i == HP - 1:
                    wp_dma(1)
                if 1 <= i <= HP:
                    attn_core(i - 1)
                if i < HP:
                    qproj(i)
                if i == HP:
                    # fill the norm-chain tail with the first out-proj
                    # accumulations over the already-normalized head pairs
                    for tt in range(2):
                        po = spp.tile([128, 512], F32, tag="sp")
                        for hp in range(HP - 2):
                            nc.tensor.matmul(
                                po[:, :],
                                lhsT=ctx_sb[:, hp * TQ + tt * 128:
                                            hp * TQ + tt * 128 + 128],
                                rhs=wblks[(0, hp // 8)][:, (hp % 8) * 512:
                                                        (hp % 8) * 512 + 512],
                                start=(hp == 0), stop=False)
                        pos[tt] = po
                if i >= 2:
                    norm(i - 2)

        # ============ phase E: output projection ==========================
        with (
            tc.tile_pool(name="osb", bufs=3) as osb,
        ):
            for ob in range(QC):
                if ob >= 2:
                    wp_dma(ob)
                for tt in range(TQ // 128):
                    if ob == 0 and tt in pos:
                        po = pos[tt]
                        hps = range(HP - 2, HP)
                    else:
                        po = spp.tile([128, 512], F32, tag="sp")
                        hps = range(HP)
                    for hp in hps:
                        nc.tensor.matmul(
                            po[:, :],
                            lhsT=ctx_sb[:, hp * TQ + tt * 128:
                                        hp * TQ + tt * 128 + 128],
                            rhs=wblks[(ob, hp // 8)][:, (hp % 8) * 512:
                                                     (hp % 8) * 512 + 512],
                            start=(hp == 0 and (ob != 0 or tt not in pos)),
                            stop=(hp == HP - 1))
                    ot = osb.tile([128, 512], F32, tag="outsb")
                    nc.vector.tensor_copy(ot[:, :], po[:, :])
                    nc.sync.dma_start(
                        out=OUT[tt * 128:(tt + 1) * 128,
                                ob * 512:(ob + 1) * 512],
                        in_=ot[:, :])
        wst2.release()
        wqs.release()


# ---------------------------------------------------------------------------
# host-side data prep
# ---------------------------------------------------------------------------

def _tile_weight_T(w, nt):
    # w [nt*128, 2048] -> [128, nt*DMT*128]:
    # [p][t, k, o] = w[128*t + o, 128*k + p]
    return np.ascontiguousarray(
        w.reshape(nt, 128, DMT, 128).transpose(3, 0, 2, 1).reshape(128, -1)
    ).astype(BF_NP)


def _tile_weight_kv(w):
    # w [512, 2048] -> [128, DMT*512]: [p][k,o] = w[o, dmt*128+p]
    return np.ascontiguousarray(
        w.reshape(512, DMT, 128).transpose(2, 1, 0).reshape(128, -1)
    ).astype(BF_NP)


def _tile_weight_q(w):
    # w [2048, 2048] -> [128, QC*DMT*512]: [p][qc,dmt,o] = w[qc*512+o, dmt*128+p]
    return np.ascontiguousarray(
        w.reshape(QC, 512, DMT, 128).transpose(3, 0, 2, 1).reshape(128, -1)
    ).astype(BF_NP)


def _tile_x(xt):
    # xt [ntok, 2048] -> [128, DMT*ntok]: [p][k,t] = xt[t, 128k+p]
    n = xt.shape[0]
    return np.ascontiguousarray(
        xt.T.reshape(DMT, 128, n).transpose(1, 0, 2).reshape(128, -1)
    ).astype(BF_NP)


def _rope_tables_T(pos):
    # transposed-layout tables [128, len(pos)]: row r = 64*a + d (a head in
    # pair, d dim); cos[r, t] = cos(pos_t * invf[d % 32]);
    # ssin[r, t] = sin(...) * (-1 if d < 32 else +1)
    invf = 1.0 / (ROPE_BASE ** (np.arange(0, D_HEAD, 2, dtype=np.float64)
                                / D_HEAD))
    d = np.arange(128) % 64
    ang = pos[None, :] * invf[d % 32][:, None]          # [128, n]
    sign = np.where(d < 32, -1.0, 1.0)[:, None]
    cos = np.cos(ang).astype(BF_NP)
    ssin = (np.sin(ang) * sign).astype(BF_NP)
    return np.ascontiguousarray(cos), np.ascontiguousarray(ssin)


# data-masked block list: column m of the mask tensor covers (tc_m, j_m)
_MASK_POS = [(0, 0), (0, 1), (0, 2), (0, 3),
             (1, 4), (1, 5), (1, 6), (1, 7), (0, 8)]


def _core_inputs(x, shared, b, c):
    qoff = c * TQ
    xq = x[b, qoff:qoff + TQ]
    xkv = np.concatenate([x[b, :SINK], x[b, T - WINDOW:]], 0)

    qpos = (qoff + np.arange(TQ)).astype(np.float64)
    kpos = np.concatenate([np.arange(SINK),
                           np.arange(T - WINDOW, T)]).astype(np.float64)
    cosq, ssinq = _rope_tables_T(qpos)
    cosk, ssink = _rope_tables_T(kpos)

    # exp bias [128, tc*9+j]: -1e30 where the whole row is masked
    p = np.arange(128)
    bias = np.zeros((128, 2 * ST), np.float32)
    for tcq in range(TC):
        for j in range(ST):
            kill = (128 * j + p > qoff + 512 * tcq + 511) | (128 * j + p >= S)
            bias[kill, tcq * ST + j] = NEG

    # data mask [128, 9 positions, 512]
    mask = np.zeros((128, 9, 512), BF_NP)
    t = np.arange(512)
    for m, (tcq, j) in enumerate(_MASK_POS):
        keep = (qoff + 512 * tcq + t[None, :]) >= (128 * j + p[:, None])
        mask[:, m, :] = keep.astype(BF_NP)

    d = {
        "xq_t": _tile_x(xq),
        "xkv_t": _tile_x(xkv),
        "cosq_t": cosq, "ssinq_t": ssinq,
        "cosk_t": cosk, "ssink_t": ssink,
        "bias_t": bias,
        "mask_t": np.ascontiguousarray(mask.reshape(128, -1)),
    }
    d.update(shared)
    return d


def _prep_all(x, wq, wk, wv, w_proj):
    # dens rows: row 0 = den(head B), row 1 = den(head A) (stage swizzle)
    sel = np.zeros((2, 128), dtype=BF_NP)
    sel[0, 64:128] = 1
    sel[1, 0:64] = 1
    swap = np.zeros((128, 128), dtype=BF_NP)
    for cc in range(128):
        swap[cc, 64 * (cc // 64) + (cc % 64 + 32) % 64] = 1
    shared = {
        "wq_t": _tile_weight_T(wq, HP),
        "wk_t": _tile_weight_T(wk, KT),
        "wv_t": _tile_weight_kv(wv),
        "wp_t": _tile_weight_q(w_proj),
        "sel_t": sel,
        "swap_t": swap,
    }
    return [_core_inputs(x, shared, *divmod(core, 4)) for core in range(NCORES)]


def _get_nc():
    if "nc" not in _CACHED:
        _CACHED["nc"] = _build_bass()
    return _CACHED["nc"]


def _run(x, wq, wk, wv, w_proj, trace=False, **kw):
    nc = _get_nc()
    in_maps = _prep_all(np.asarray(x, np.float32), np.asarray(wq, np.float32),
                        np.asarray(wk, np.float32), np.asarray(wv, np.float32),
                        np.asarray(w_proj, np.float32))
    res = run_bass_kernel_spmd(nc, in_maps, list(range(NCORES)), trace=trace,
                               **kw)
    out = np.empty((B, T, D_MODEL), np.float32)
    for core in range(NCORES):
        b, c = divmod(core, 4)
        out[b, c * TQ:(c + 1) * TQ] = np.asarray(res.results[core]["out"],
                                                 np.float32)
    return out, res


def kernel(x, wq, wk, wv, w_proj):
    out, _ = _run(x, wq, wk, wv, w_proj)
    return out


# revision 37
# speedup vs baseline: 1.0367x; 1.0259x over previous
"""GQA attention with sliding-window+sink KV slicing on 8 trn2 NeuronCores.

Sharding: core = (batch b in 0..1, query-chunk c in 0..3); each core handles
1024 query tokens of one batch against the full sliced KV (sink 4 + window
1024 = 1028 positions), weights replicated.  No collectives; host concats.

v2 restructure (vs v0 phase-serial kernel):
  * K/V projections computed DIRECTLY TRANSPOSED (lhsT = W^T tiles, rhs =
    x^T) -- no PE transposes.  RoPE in the transposed [d, t] layout via a
    32-row-block swap matmul on PE + 3 bf16 DVE ops with sign-baked sin
    tables.
  * Q projection + RoPE fused INTO the attention head-pair loop (software
    pipelined one iteration ahead), so PE stays busy during the ACT-bound
    exp stretches and the HAM clock never re-throttles.
  * Causal masking split: fully-masked (tc,j) blocks are killed for free by
    a per-partition -1e30 bias table fed to the exp activation; only the 9
    diagonal-capable blocks get a DVE mask multiply (vs 18 before).
  * Softmax denominator via the ones-column-in-V trick (row 64 of the PV
    accumulator); per-head-pair normalization with reciprocal_approx_fast
    and a tiny selector matmul, all inside the loop.
  * Output projection last, PE-dense, weights streamed.
"""

import numpy as np
import ml_dtypes

import concourse.bass as bass
import concourse.bacc as bacc
import concourse.tile as tile
import concourse.mybir as mybir
from concourse.bass_utils import run_bass_kernel_spmd

BF = mybir.dt.bfloat16
F32 = mybir.dt.float32
BF_NP = ml_dtypes.bfloat16

# problem constants
D_MODEL = 2048
N_HEADS = 32
N_KV = 8
D_HEAD = 64
GROUP = 4
B, T = 2, 4096
WINDOW = 1024
SINK = 4
ROPE_BASE = 10000.0

# sharding/tiling constants
NCORES = 8
TQ = 1024             # query tokens per core
S = SINK + WINDOW     # 1028 kv positions
DMT = D_MODEL // 128  # 16 contraction tiles
TC = TQ // 512        # 2 query 512-chunks
QC = D_MODEL // 512   # 4 out-proj 512-chunks
ST = (S + 127) // 128  # 9 s-tiles (last has 4 rows)
HP = N_HEADS // 2     # 16 head pairs
KT = N_KV // 2        # 4 kv-head pair tiles
SCALE = float(1.0 / np.sqrt(D_HEAD))
NEG = -1.0e30
F8 = mybir.dt.float8e4
F8_NP = ml_dtypes.float8_e4m3fn
WSC = 64.0              # fp8 weight pre-scale for wq/wk (and wv, folded
                        # into wp host-side); un-done in the exp scale
ESCALE = SCALE / (WSC * WSC)  # exp scale: q,k each carry a WSC factor
DR = mybir.MatmulPerfMode.DoubleRow

_CACHED = {}


SP8 = 1040  # fp8 xkv dm-stride, padded so the DoubleRow pair-step is 16-aligned


def _sp(j):
    return 128 if j < ST - 1 else S - 128 * (ST - 1)


def _ins0(ap, dim_idx, n):
    """Copy of `ap` with a step-0 (broadcast) dim inserted at free-dim
    position `dim_idx` (0 = right after the partition dim)."""
    dims = list(ap.ap)
    dims.insert(1 + dim_idx, [0, n])
    return bass.AP(tensor=ap.tensor, offset=ap.offset, ap=dims)


def _build_bass():
    nc = bacc.Bacc("TRN2", target_bir_lowering=False, debug=False,
                   num_devices=NCORES)

    def din(name, shape, dt=BF):
        return nc.dram_tensor(name, shape, dt, kind="ExternalInput").ap()

    XQ = din("xq_t", [128, DMT * TQ])
    XKV = din("xkv_t", [128, DMT * S])
    WQ = din("wq_t", [128, HP * DMT * 128])
    WK = din("wk_t", [128, KT * DMT * 128])
    WV = din("wv_t", [128, DMT * 512])
    WP = din("wp_t", [128, QC * DMT * 512])
    COSQ = din("cosq_t", [128, TQ])
    SSINQ = din("ssinq_t", [128, TQ])
    COSK = din("cosk_t", [128, S])
    SSINK = din("ssink_t", [128, S])
    BIAS = din("bias_t", [128, 2 * ST], F32)
    MASK = din("mask_t", [128, 9 * 512])
    SWAP = din("swap_t", [128, 128])
    SEL = din("sel_t", [2, 128])
    OUT = nc.dram_tensor("out", [TQ, D_MODEL], F32, kind="ExternalOutput").ap()

    with tile.TileContext(nc) as tc:
        _body(tc, XQ, XKV, WQ, WK, WV, WP, COSQ, SSINQ, COSK, SSINK, BIAS,
              MASK, SWAP, SEL, OUT)
    nc.compile()
    return nc


def _body(tc, XQ, XKV, WQ, WK, WV, WP, COSQ, SSINQ, COSK, SSINK, BIAS,
          MASK, SWAP, SEL, OUT):
    nc = tc.nc

    def load(pool, name, src, shape, dt=BF):
        t = pool.tile(shape, dt, tag=name, name=name)
        nc.sync.dma_start(out=t[:, :], in_=src)
        return t

    with (
        tc.tile_pool(name="life", bufs=1) as life,
        tc.tile_pool(name="spp", bufs=2, space="PSUM") as spp,
        tc.tile_pool(name="psc", bufs=2, space="PSUM") as psc,
        tc.tile_pool(name="ppv", bufs=1, space="PSUM") as ppv,
    ):
        kT_sb = life.tile([128, N_KV * S], BF, tag="kT")   # [kd dup-halves, s]
        v_sb = life.tile([128, ST * N_KV * 65 + 63], BF, tag="v")
        ctx_sb = life.tile([128, HP * TQ], BF, tag="ctx")  # [dm, t]
        swap_sb = load(life, "swap_sb", SWAP, [128, 128])
        sel_sb = load(life, "sel_sb", SEL, [2, 128])
        bias_sb = load(life, "bias_sb", BIAS, [128, 2 * ST], F32)
        mask_sb = load(life, "mask_sb", MASK, [128, 9 * 512])
        cosq_sb = load(life, "cosq_sb", COSQ, [128, TQ])
        ssinq_sb = load(life, "ssinq_sb", SSINQ, [128, TQ])
        wblks = {}

        # ============ phase B: K/V projection (transposed) ================
        with (
            tc.tile_pool(name="p2s", bufs=1) as p2s,
            tc.tile_pool(name="kst", bufs=2) as kst,
        ):
            wk_sb = p2s.tile([128, KT * DMT * 128], BF, tag="wk_sb")
            for c in range(2):
                h = KT * DMT * 64
                nc.sync.dma_start(out=wk_sb[:, c * h:(c + 1) * h],
                                  in_=WK[:, c * h:(c + 1) * h])
            cosk_sb = load(p2s, "cosk_sb", COSK, [128, S])
            ssink_sb = load(p2s, "ssink_sb", SSINK, [128, S])
            xkv_sb = p2s.tile([128, DMT * S], BF, tag="xkv_sb")
            for c in range(4):
                nc.sync.dma_start(
                    out=xkv_sb[:, c * 4 * S:(c + 1) * 4 * S],
                    in_=XKV[:, c * 4 * S:(c + 1) * 4 * S])
            wv_sb = load(p2s, "wv_sb", WV, [128, DMT * 512])
            xq_sb = life.tile([128, DMT * TQ], BF, tag="xq_sb")
            for c in range(4):
                nc.sync.dma_start(
                    out=xq_sb[:, c * 4 * TQ:(c + 1) * 4 * TQ],
                    in_=XQ[:, c * 4 * TQ:(c + 1) * 4 * TQ])
            xkv3 = xkv_sb[:, :].rearrange("p (k s) -> p k s", k=DMT)
            wk3 = wk_sb[:, :].rearrange("p (k o) -> p k o", k=KT * DMT)
            wv3 = wv_sb[:, :].rearrange("p (k o) -> p k o", k=DMT)

            SCH = [(0, 512), (512, 512), (1024, 4)]
            for ot in range(KT):
                # kT o-tile [128 = heads (2ot, 2ot+1) x 64d, s]
                ka = kst.tile([128, S], BF, tag="ka")
                for off, w in SCH:
                    pk = spp.tile([128, 512], F32, tag="sp")
                    for dm in range(DMT):
                        nc.tensor.matmul(
                            pk[:, :w],
                            lhsT=wk3[:, ot * DMT + dm, :],
                            rhs=xkv3[:, dm, off:off + w],
                            start=(dm == 0), stop=(dm == DMT - 1))
                    nc.vector.tensor_copy(ka[:, off:off + w], pk[:, :w])
                kb = kst.tile([128, S], BF, tag="kb")
                for off, w in SCH:
                    ps = spp.tile([128, 512], F32, tag="sp")
                    nc.tensor.matmul(ps[:, :w], lhsT=swap_sb[:, :],
                                     rhs=ka[:, off:off + w],
                                     start=True, stop=True)
                    nc.vector.tensor_copy(kb[:, off:off + w], ps[:, :w])
                kc = kst.tile([128, S], BF, tag="kc")
                nc.vector.tensor_mul(kc[:, :], ka[:, :], cosk_sb[:, :])
                nc.vector.tensor_mul(kb[:, :], kb[:, :], ssink_sb[:, :])
                gA, gB = 2 * ot, 2 * ot + 1
                nc.vector.tensor_add(kT_sb[0:64, gA * S:(gA + 1) * S],
                                     kc[0:64, :], kb[0:64, :])
                nc.vector.tensor_add(kT_sb[64:128, gB * S:(gB + 1) * S],
                                     kc[64:128, :], kb[64:128, :])
            # duplicate each kv head's k rows into the other partition half
            for g in range(N_KV):
                if g % 2 == 0:
                    nc.sync.dma_start(out=kT_sb[64:128, g * S:(g + 1) * S],
                                      in_=kT_sb[0:64, g * S:(g + 1) * S])
                else:
                    nc.sync.dma_start(out=kT_sb[0:64, g * S:(g + 1) * S],
                                      in_=kT_sb[64:128, g * S:(g + 1) * S])

            # V projection (+ ones columns), s-major as before
            nc.vector.memset(v_sb[:, :], 0.0)
            v4 = v_sb[:, 0:ST * N_KV * 65].rearrange("p (s h c) -> p s h c", s=ST, c=65)
            for ss in range(ST):
                sp = _sp(ss)
                pv = spp.tile([128, 512], F32, tag="sp")
                for dm in range(DMT):
                    nc.tensor.matmul(
                        pv[:sp, :],
                        lhsT=xkv3[:, dm, ss * 128:ss * 128 + sp],
                        rhs=wv3[:, dm, :],
                        start=(dm == 0), stop=(dm == DMT - 1))
                nc.vector.tensor_copy(
                    v4[:sp, ss, :, 0:64],
                    pv[:sp, :].rearrange("p (h d) -> p h d", h=8))
                nc.vector.memset(v4[:sp, ss, :, 64:65], 1.0)

        # ============ fused Q-proj + attention head-pair loop =============
        wst2 = tc.alloc_tile_pool(name="wst2", bufs=2)

        def wp_dma(ob):
            for h in range(2):
                wb = wst2.tile([128, 8 * 512], BF, tag="wblk2", bufs=3,
                               name=f"wb{ob}{h}")
                nc.sync.dma_start(
                    out=wb[:, :],
                    in_=WP[:, (ob * DMT + h * 8) * 512:
                            (ob * DMT + h * 8 + 8) * 512])
                wblks[(ob, h)] = wb

        with (
            tc.tile_pool(name="wqs", bufs=2) as wqs,
            tc.tile_pool(name="qtp", bufs=2) as qtp,
            tc.tile_pool(name="prb", bufs=2) as prb,
            tc.tile_pool(name="stg", bufs=2) as stg,
            tc.tile_pool(name="dnp", bufs=2) as dnp,
        ):
            mask3 = mask_sb[:, :].rearrange("p (m v) -> p m v", m=9)
            qts = [None] * HP
            wqh = [None] * HP
            denh = [None] * HP

            def wq_dma(hp):
                wq_hp = wqs.tile([128, DMT * 128], BF, tag="wq")
                nc.sync.dma_start(
                    out=wq_hp[:, :],
                    in_=WQ[:, hp * DMT * 128:(hp + 1) * DMT * 128])
                wqh[hp] = wq_hp

            xq3 = xq_sb[:, :].rearrange("p (k t) -> p k t", k=DMT)

            def qproj(hp):
                wq3 = wqh[hp][:, :].rearrange("p (k o) -> p k o", k=DMT)
                qa = qtp.tile([128, TQ], BF, tag="qa")
                for ch in range(TC):
                    pq = spp.tile([128, 512], F32, tag="sp")
                    for dm in range(DMT):
                        nc.tensor.matmul(
                            pq[:, :],
                            lhsT=wq3[:, dm, :],
                            rhs=xq3[:, dm, ch * 512:ch * 512 + 512],
                            start=(dm == 0), stop=(dm == DMT - 1))
                    nc.vector.tensor_copy(qa[:, ch * 512:(ch + 1) * 512],
                                          pq[:, :])
                qb = qtp.tile([128, TQ], BF, tag="qb", bufs=1)
                for ch in range(TC):
                    ps = spp.tile([128, 512], F32, tag="sp")
                    nc.tensor.matmul(ps[:, :], lhsT=swap_sb[:, :],
                                     rhs=qa[:, ch * 512:(ch + 1) * 512],
                                     start=True, stop=True)
                    nc.vector.tensor_copy(qb[:, ch * 512:(ch + 1) * 512],
                                          ps[:, :])
                qt = qtp.tile([128, TQ], BF, tag="qt")
                nc.vector.tensor_mul(qt[:, :], qa[:, :], cosq_sb[:, :])
                nc.vector.tensor_mul(qb[:, :], qb[:, :], ssinq_sb[:, :])
                nc.vector.tensor_add(qt[:, :], qt[:, :], qb[:, :])
                qts[hp] = qt

            def attn_core(hp):
                qt = qts[hp]
                g = hp // 2
                dens_hp = dnp.tile([2, TQ], BF, tag="dh", bufs=2)
                denh[hp] = dens_hp
                for tcq in range(TC):
                    probs = prb.tile([128, ST * 1024], BF, tag="probs")
                    p4 = probs[:, :].rearrange("p (s u v) -> p s u v",
                                               s=ST, u=2)
                    for j in range(ST):
                        sp = _sp(j)
                        sc = psc.tile([128, 1024], F32, tag="sc")
                        nc.tensor.matmul(
                            sc[:sp, 0:512],
                            lhsT=kT_sb[0:64, g * S + j * 128:
                                       g * S + j * 128 + sp],
                            rhs=qt[0:64, tcq * 512:tcq * 512 + 512],
                            start=True, stop=True, tile_position=(0, 0))
                        nc.tensor.matmul(
                            sc[:sp, 512:1024],
                            lhsT=kT_sb[64:128, g * S + j * 128:
                                       g * S + j * 128 + sp],
                            rhs=qt[64:128, tcq * 512:tcq * 512 + 512],
                            start=True, stop=True, tile_position=(64, 0))
                        nc.scalar.activation(
                            p4[:sp, j, :, :],
                            sc[:sp, :].rearrange("p (u v) -> p u v", u=2),
                            mybir.ActivationFunctionType.Exp, scale=SCALE,
                            bias=bias_sb[:sp, tcq * ST + j:tcq * ST + j + 1])
                    # data-mask only diagonal-capable blocks; the rest were
                    # killed (or kept) by the exp bias
                    if tcq == 0:
                        nc.vector.tensor_mul(
                            p4[:, 0:4, :, :], p4[:, 0:4, :, :],
                            _ins0(mask3[:, 0:4, :], 1, 2))
                        spl = _sp(ST - 1)
                        nc.vector.tensor_mul(
                            p4[:spl, 8, :, :], p4[:spl, 8, :, :],
                            _ins0(mask3[:spl, 8, :], 0, 2))
                    else:
                        nc.vector.tensor_mul(
                            p4[:, 4:8, :, :], p4[:, 4:8, :, :],
                            _ins0(mask3[:, 4:8, :], 1, 2))
                    # PV: ctx_aug[65, t] per head; denominator in row 64
                    pvt = ppv.tile([128, 1024], F32, tag="pv")
                    for j in range(ST):
                        sp = _sp(j)
                        vw = v_sb[:sp, (j * N_KV + g) * 65:
                                  (j * N_KV + g) * 65 + 128]
                        nc.tensor.matmul(
                            pvt[0:128, 0:512],
                            lhsT=vw,
                            rhs=p4[:sp, j, 0, :],
                            start=(j == 0), stop=(j == ST - 1))
                        nc.tensor.matmul(
                            pvt[0:128, 512:1024],
                            lhsT=vw,
                            rhs=p4[:sp, j, 1, :],
                            start=(j == 0), stop=(j == ST - 1))
                    # ctx head A plain copy (same partitions)
                    nc.vector.tensor_copy(
                        ctx_sb[0:64, hp * TQ + tcq * 512:
                               hp * TQ + tcq * 512 + 512],
                        pvt[0:64, 0:512])
                    # head B ctx + dens staged; DMA shifts partitions
                    stb = stg.tile([128, 1024], BF, tag="stb")
                    nc.vector.tensor_copy(stb[0:65, 0:512],
                                          pvt[0:65, 512:1024])
                    nc.vector.tensor_copy(stb[64:65, 512:1024],
                                          pvt[64:65, 0:512])
                    nc.sync.dma_start(
                        out=ctx_sb[64:128, hp * TQ + tcq * 512:
                                   hp * TQ + tcq * 512 + 512],
                        in_=stb[0:64, 0:512])
                    nc.sync.dma_start(
                        out=dens_hp[0:2, tcq * 512:(tcq + 1) * 512],
                        in_=stb[64:65, 0:1024])
                qts[hp] = None

            def norm(hp):
                # normalize this head pair's ctx (decoupled by 2 iterations
                # so the reciprocal chain never stalls the score pipeline)
                dens_hp = denh[hp]
                densf = dnp.tile([2, TQ], F32, tag="df", bufs=1)
                nc.vector.tensor_copy(densf[:, :], dens_hp[:, :])
                rf = dnp.tile([2, TQ], F32, tag="rf", bufs=1)
                nc.vector.reciprocal_approx_fast(rf[:, :], densf[:, :])
                rb = dnp.tile([2, TQ], BF, tag="rb", bufs=1)
                nc.vector.tensor_copy(rb[:, :], rf[:, :])
                pr = psc.tile([128, 1024], F32, tag="sc")
                for ch in range(TC):
                    nc.tensor.matmul(pr[:, ch * 512:(ch + 1) * 512],
                                     lhsT=sel_sb[:, :],
                                     rhs=rb[:, ch * 512:(ch + 1) * 512],
                                     start=True, stop=True)
                csl = ctx_sb[:, hp * TQ:(hp + 1) * TQ]
                nc.vector.tensor_mul(csl, csl, pr[:, :])
                denh[hp] = None

            pos = {}
            for i in range(HP + 2):
                if i < HP:
                    wq_dma(i)
                if i == HP - 2:
                    wp_dma(0)
                if i == HP - 1:
                    wp_dma(1)
                if 1 <= i <= HP:
                    attn_core(i - 1)
                if i < HP:
                    qproj(i)
                if i == HP:
                    # fill the norm-chain tail with the first out-proj
                    # accumulations over the already-normalized head pairs
                    for tt in range(2):
                        po = spp.tile([128, 512], F32, tag="sp")
                        for hp in range(HP - 2):
                            nc.tensor.matmul(
                                po[:, :],
                                lhsT=ctx_sb[:, hp * TQ + tt * 128:
                                            hp * TQ + tt * 128 + 128],
                                rhs=wblks[(0, hp // 8)][:, (hp % 8) * 512:
                                                        (hp % 8) * 512 + 512],
                                start=(hp == 0), stop=False)
                        pos[tt] = po
                if i >= 2:
                    norm(i - 2)

        # ============ phase E: output projection ==========================
        with (
            tc.tile_pool(name="osb", bufs=3) as osb,
        ):
            for ob in range(QC):
                if ob >= 2:
                    wp_dma(ob)
                for tt in range(TQ // 128):
                    if ob == 0 and tt in pos:
                        po = pos[tt]
                        hps = range(HP - 2, HP)
                    else:
                        po = spp.tile([128, 512], F32, tag="sp")
                        hps = range(HP)
                    for hp in hps:
                        nc.tensor.matmul(
                            po[:, :],
                            lhsT=ctx_sb[:, hp * TQ + tt * 128:
                                        hp * TQ + tt * 128 + 128],
                            rhs=wblks[(ob, hp // 8)][:, (hp % 8) * 512:
                                                     (hp % 8) * 512 + 512],
                            start=(hp == 0 and (ob != 0 or tt not in pos)),
                            stop=(hp == HP - 1))
                    ot = osb.tile([128, 512], F32, tag="outsb")
                    nc.vector.tensor_copy(ot[:, :], po[:, :])
                    nc.sync.dma_start(
                        out=OUT[tt * 128:(tt + 1) * 128,
                                ob * 512:(ob + 1) * 512],
                        in_=ot[:, :])
        wst2.release()


# ---------------------------------------------------------------------------
# host-side data prep
# ---------------------------------------------------------------------------

def _tile_weight_T(w, nt):
    # w [nt*128, 2048] -> [128, nt*DMT*128]:
    # [p][t, k, o] = w[128*t + o, 128*k + p]
    return np.ascontiguousarray(
        w.reshape(nt, 128, DMT, 128).transpose(3, 0, 2, 1).reshape(128, -1)
    ).astype(BF_NP)


def _tile_weight_kv(w):
    # w [512, 2048] -> [128, DMT*512]: [p][k,o] = w[o, dmt*128+p]
    return np.ascontiguousarray(
        w.reshape(512, DMT, 128).transpose(2, 1, 0).reshape(128, -1)
    ).astype(BF_NP)


def _tile_weight_q(w):
    # w [2048, 2048] -> [128, QC*DMT*512]: [p][qc,dmt,o] = w[qc*512+o, dmt*128+p]
    return np.ascontiguousarray(
        w.reshape(QC, 512, DMT, 128).transpose(3, 0, 2, 1).reshape(128, -1)
    ).astype(BF_NP)


def _tile_x(xt):
    # xt [ntok, 2048] -> [128, DMT*ntok]: [p][k,t] = xt[t, 128k+p]
    n = xt.shape[0]
    return np.ascontiguousarray(
        xt.T.reshape(DMT, 128, n).transpose(1, 0, 2).reshape(128, -1)
    ).astype(BF_NP)


def _rope_tables_T(pos):
    # transposed-layout tables [128, len(pos)]: row r = 64*a + d (a head in
    # pair, d dim); cos[r, t] = cos(pos_t * invf[d % 32]);
    # ssin[r, t] = sin(...) * (-1 if d < 32 else +1)
    invf = 1.0 / (ROPE_BASE ** (np.arange(0, D_HEAD, 2, dtype=np.float64)
                                / D_HEAD))
    d = np.arange(128) % 64
    ang = pos[None, :] * invf[d % 32][:, None]          # [128, n]
    sign = np.where(d < 32, -1.0, 1.0)[:, None]
    cos = np.cos(ang).astype(BF_NP)
    ssin = (np.sin(ang) * sign).astype(BF_NP)
    return np.ascontiguousarray(cos), np.ascontiguousarray(ssin)


# data-masked block list: column m of the mask tensor covers (tc_m, j_m)
_MASK_POS = [(0, 0), (0, 1), (0, 2), (0, 3),
             (1, 4), (1, 5), (1, 6), (1, 7), (0, 8)]


def _core_inputs(x, shared, b, c):
    qoff = c * TQ
    xq = x[b, qoff:qoff + TQ]
    xkv = np.concatenate([x[b, :SINK], x[b, T - WINDOW:]], 0)

    qpos = (qoff + np.arange(TQ)).astype(np.float64)
    kpos = np.concatenate([np.arange(SINK),
                           np.arange(T - WINDOW, T)]).astype(np.float64)
    cosq, ssinq = _rope_tables_T(qpos)
    cosk, ssink = _rope_tables_T(kpos)

    # exp bias [128, tc*9+j]: -1e30 where the whole row is masked
    p = np.arange(128)
    bias = np.zeros((128, 2 * ST), np.float32)
    for tcq in range(TC):
        for j in range(ST):
            kill = (128 * j + p > qoff + 512 * tcq + 511) | (128 * j + p >= S)
            bias[kill, tcq * ST + j] = NEG

    # data mask [128, 9 positions, 512]
    mask = np.zeros((128, 9, 512), BF_NP)
    t = np.arange(512)
    for m, (tcq, j) in enumerate(_MASK_POS):
        keep = (qoff + 512 * tcq + t[None, :]) >= (128 * j + p[:, None])
        mask[:, m, :] = keep.astype(BF_NP)

    d = {
        "xq_t": _tile_x(xq),
        "xkv_t": _tile_x(xkv),
        "cosq_t": cosq, "ssinq_t": ssinq,
        "cosk_t": cosk, "ssink_t": ssink,
        "bias_t": bias,
        "mask_t": np.ascontiguousarray(mask.reshape(128, -1)),
    }
    d.update(shared)
    return d


def _prep_all(x, wq, wk, wv, w_proj):
    # dens rows: row 0 = den(head B), row 1 = den(head A) (stage swizzle)
    sel = np.zeros((2, 128), dtype=BF_NP)
    sel[0, 64:128] = 1
    sel[1, 0:64] = 1
    swap = np.zeros((128, 128), dtype=BF_NP)
    for cc in range(128):
        swap[cc, 64 * (cc // 64) + (cc % 64 + 32) % 64] = 1
    shared = {
        "wq_t": _tile_weight_T(wq, HP),
        "wk_t": _tile_weight_T(wk, KT),
        "wv_t": _tile_weight_kv(wv),
        "wp_t": _tile_weight_q(w_proj),
        "sel_t": sel,
        "swap_t": swap,
    }
    return [_core_inputs(x, shared, *divmod(core, 4)) for core in range(NCORES)]


def _get_nc():
    if "nc" not in _CACHED:
        _CACHED["nc"] = _build_bass()
    return _CACHED["nc"]


def _run(x, wq, wk, wv, w_proj, trace=False, **kw):
    nc = _get_nc()
    in_maps = _prep_all(np.asarray(x, np.float32), np.asarray(wq, np.float32),
                        np.asarray(wk, np.float32), np.asarray(wv, np.float32),
                        np.asarray(w_proj, np.float32))
    res = run_bass_kernel_spmd(nc, in_maps, list(range(NCORES)), trace=trace,
                               **kw)
    out = np.empty((B, T, D_MODEL), np.float32)
    for core in range(NCORES):
        b, c = divmod(core, 4)
        out[b, c * TQ:(c + 1) * TQ] = np.asarray(res.results[core]["out"],
                                                 np.float32)
    return out, res


def kernel(x, wq, wk, wv, w_proj):
    out, _ = _run(x, wq, wk, wv, w_proj)
    return out
